# revision 1
# baseline (speedup 1.0000x reference)
"""Swin-style window attention (B=1024 windows, N=64 tokens, DIM=768, 12 heads)
for 8 Trainium2 NeuronCores.

Strategy: data-parallel over windows (128 windows/core). Everything is
feature-major on device; the host pre-transposes x and post-transposes the
output. Per core:
  - qk^T = (Wqk^T x^T + bqk) in fp32r (full-rate), cast bf16
  - V = x Wv token-major in fp32r, cast bf16
  - per window-pair: S = q.k^T + rel-pos-bias (PSUM accumulation; bias added
    via identity matmul), softmax along free axis (max-subtracted, exp on ACT,
    grouped sums on DVE, normalize on GPSIMD), P^T via PE transposes,
    O = P V token-major (diagonal PE quadrants), O^T via PE transposes
  - out^T = proj_w^T O^T + proj_b in bf16, fp32 out, DMA to HBM
Host gathers the 8 outputs and transposes back.

All matmul quadrant pairs use diagonal tile_position only: concurrent
matmuls with overlapping output partition groups but different row groups
fault the PSUM write port (verified empirically).

The local walrus accepts at most ONE semaphore wait per instruction;
split_multi_waits() hoists extra waits onto same-engine NoOps.
"""
import os
import sys

if "/opt/trn_rl_repo" not in sys.path:
    sys.path.insert(0, "/opt/trn_rl_repo")

import numpy as np
import ml_dtypes

import concourse.bass as bass
import concourse.tile as tile
from concourse import mybir
from concourse.bass_utils import run_bass_kernel_spmd

DIM = 768
HEADS = 12
N = 64            # tokens per window
B = 1024          # windows
NCORES = 8
BC = B // NCORES          # windows per core = 128
TOK = BC * N              # tokens per core = 8192
CHTOK = 512               # tokens per chunk
NCHUNK = TOK // CHTOK     # 16
WPC = CHTOK // 128        # window pairs per chunk = 4
KC = DIM // 128           # 6 contraction chunks
SCALE = (DIM // HEADS) ** -0.5

F32 = mybir.dt.float32
F32R = mybir.dt.float32r
BF16 = mybir.dt.bfloat16
AF = mybir.ActivationFunctionType
ALU = mybir.AluOpType
AX = mybir.AxisListType

_CACHE = {}


def _split_multi_waits(nc, limit=1):
    """Walrus here encodes at most `limit` sem-waits per instruction; hoist
    extras onto preceding same-engine NoOps (engine streams are in-order)."""
    ctr = 0
    for fn in nc.m.functions:
        for blk in fn.blocks:
            insts = list(blk.instructions)
            out = []
            changed = False
            for inst in insts:
                si = inst.sync_info
                waits = list(si.on_wait) if si is not None else []
                if len(waits) > limit:
                    changed = True
                    extra, keep = waits[:-limit], waits[-limit:]
                    for i in range(0, len(extra), limit):
                        nop = mybir.InstNoOp(name=f"WSPLIT-{ctr}", ins=[], outs=[])
                        ctr += 1
                        nop.engine = inst.engine
                        nop.sync_info = mybir.SyncInfo(
                            on_wait=extra[i:i + limit], on_update=[])
                        nc.register_instruction(nop)
                        out.append(nop)
                    si.on_wait = keep
                out.append(inst)
            if changed:
                while len(blk.instructions):
                    blk.instructions.pop()
                for inst in out:
                    blk.instructions.append(inst)
    return ctr


def _bcast_free(ap, n):
    """AP view broadcasting a [P, G] tile to [P, G, n] via zero-stride."""
    return bass.AP(tensor=ap.tensor, offset=ap.offset,
                   ap=[list(ap.ap[0]), list(ap.ap[1]), [0, n]])


def _build(safe_softmax=False):
    import contextlib

    PDT = BF16 if int(os.environ.get("KERNEL_BF16_PROJ", "1")) else F32R
    nc = bass.Bass()
    d_x = nc.dram_tensor("xT", [DIM, TOK], PDT, kind="ExternalInput")
    d_wqk = nc.dram_tensor("wqk", [12, KC, 128, 128], PDT, kind="ExternalInput")
    d_wv = nc.dram_tensor("wv", [DIM, DIM], PDT, kind="ExternalInput")
    d_pw = nc.dram_tensor("pw", [DIM, DIM], BF16, kind="ExternalInput")
    d_bqk = nc.dram_tensor("bqk", [128, 12], F32, kind="ExternalInput")
    d_pb = nc.dram_tensor("pb", [128, 6], F32, kind="ExternalInput")
    d_bias = nc.dram_tensor("bias", [128, DIM], BF16, kind="ExternalInput")
    d_id = nc.dram_tensor("ident", [128, 128], BF16, kind="ExternalInput")
    d_idf = nc.dram_tensor("identf", [128, 128], BF16, kind="ExternalInput")
    d_out = nc.dram_tensor("outT", [DIM, TOK], F32, kind="ExternalOutput")

    xr = d_x.rearrange("(kc p) t -> p kc t", p=128)
    wvr = d_wv.rearrange("(kc p) m -> p kc m", p=128)
    pwr = d_pw.rearrange("(kc p) m -> p kc m", p=128)
    outr = d_out.rearrange("(kc p) t -> p kc t", p=128)

    n_chunk = int(os.environ.get("KERNEL_NCHUNK", NCHUNK))
    SKIP_MAX = (not safe_softmax) and bool(int(os.environ.get("KERNEL_SKIP_MAX", "1")))
    rep = int(os.environ.get("KERNEL_REPEAT", "1"))

    with tile.TileContext(nc) as tc:
        with (
            tc.tile_pool(name="const", bufs=1) as cpool,
            tc.tile_pool(name="xin", bufs=2) as xpool,
            tc.tile_pool(name="qk", bufs=2) as qkpool,
            tc.tile_pool(name="vv", bufs=2) as vpool,
            tc.tile_pool(name="pp", bufs=4) as ppool,
            tc.tile_pool(name="ptp", bufs=4) as ptpool,
            tc.tile_pool(name="osb", bufs=4) as opool,
            tc.tile_pool(name="otc", bufs=2) as otcpool,
            tc.tile_pool(name="outp", bufs=2) as outpool,
            tc.tile_pool(name="smx", bufs=8) as smpool,
            tc.tile_pool(name="psbig", bufs=2, space="PSUM") as psbig,
            tc.tile_pool(name="pss", bufs=2, space="PSUM") as pss,
            tc.tile_pool(name="pst", bufs=1, space="PSUM") as pst,
            tc.tile_pool(name="psO", bufs=2, space="PSUM") as psO,
            tc.tile_pool(name="psot", bufs=1, space="PSUM") as psot,
        ):
            t_wqk = cpool.tile([128, 12, KC, 128], PDT)
            t_wv = cpool.tile([128, KC, DIM], PDT)
            t_pw = cpool.tile([128, KC, DIM], BF16)
            t_bqk = cpool.tile([128, 12], F32)
            t_pb = cpool.tile([128, 6], F32)
            t_bias = cpool.tile([128, DIM], BF16)
            t_id = cpool.tile([128, 128], BF16)
            t_idf = cpool.tile([128, 128], BF16)
            # smalls first, then per-mc weight blocks so the first matmul
            # group unblocks after ~0.8MB instead of the full weight load
            nc.sync.dma_start(out=t_bqk, in_=d_bqk[:, :])
            nc.sync.dma_start(out=t_bias, in_=d_bias[:, :])
            nc.sync.dma_start(out=t_id, in_=d_id[:, :])
            nc.sync.dma_start(out=t_idf, in_=d_idf[:, :])
            nc.sync.dma_start(out=t_pb, in_=d_pb[:, :])
            wqk2 = d_wqk.rearrange("mc kc p m -> p mc kc m")
            # first mc block, then chunk-0 x, then the rest of the weights:
            # the first projection group unblocks after ~1MB of DMA
            nc.sync.dma_start(out=t_wqk[:, 0, :, :], in_=wqk2[:, 0, :, :])
            t_x0 = xpool.tile([128, KC, CHTOK], PDT)
            for kc in range(KC):
                nc.sync.dma_start(out=t_x0[:, kc, :], in_=xr[:, kc, 0:CHTOK])
            for mc in range(1, 12):
                nc.sync.dma_start(out=t_wqk[:, mc, :, :], in_=wqk2[:, mc, :, :])
            for kc in range(KC):
                nc.sync.dma_start(out=t_wv[:, kc, :], in_=wvr[:, kc, :])
            for kc in range(KC):
                nc.sync.dma_start(out=t_pw[:, kc, :], in_=pwr[:, kc, :])

            skip_qkv = bool(int(os.environ.get("KERNEL_SKIP_QKV", "0")))
            skip_attn = bool(int(os.environ.get("KERNEL_SKIP_ATTN", "0")))
            skip_proj = bool(int(os.environ.get("KERNEL_SKIP_PROJ", "0")))

            def chunk_body(ch):
                c0 = ch * CHTOK
                if ch == 0:
                    t_x = t_x0
                else:
                    t_x = xpool.tile([128, KC, CHTOK], PDT)
                    for kc in range(KC):
                        nc.sync.dma_start(out=t_x[:, kc, :],
                                          in_=xr[:, kc, c0:c0 + CHTOK])

                # ---- q/k projection: qk^T [feat, tok], fp32r, -> bf16
                t_qk = qkpool.tile([128, 12, CHTOK], BF16)
                if skip_qkv:
                    nc.vector.memset(t_qk, 0.0)
                for mc in range(12 if not skip_qkv else 0):
                    ps = psbig.tile([128, CHTOK], F32, tag="big")
                    for kc in range(KC):
                        nc.tensor.matmul(
                            ps, t_wqk[:, mc, kc, :],
                            t_x[:, kc, :],
                            start=(kc == 0), stop=(kc == KC - 1))
                    nc.scalar.activation(
                        out=t_qk[:, mc, :], in_=ps, func=AF.Identity,
                        bias=t_bqk[:, mc:mc + 1], scale=1.0)

                # ---- V projection: token-major [tok, feat], fp32r -> bf16
                t_v = vpool.tile([128, WPC, DIM], BF16)
                if skip_qkv:
                    nc.vector.memset(t_v, 0.0)
                for tch in range(WPC if not skip_qkv else 0):
                    for half in range(2):
                        n0 = 384 * half
                        ps = psbig.tile([128, 384], F32, tag="big")
                        for kc in range(KC):
                            nc.tensor.matmul(
                                ps, t_x[:, kc, 128 * tch:128 * tch + 128],
                                t_wv[:, kc, n0:n0 + 384],
                                start=(kc == 0), stop=(kc == KC - 1))
                        nc.vector.tensor_copy(t_v[:, tch, n0:n0 + 384], ps)

                # ---- attention per window pair, split into half-head
                # sub-chains (heads 6g..6g+5) so S/O/T/OT are 1 PSUM bank
                # each and S/O double-buffer: deep cross-chain pipelining.
                t_ot = otcpool.tile([128, KC, CHTOK], BF16)
                if skip_attn:
                    nc.vector.memset(t_ot, 0.0)
                for wp in range(WPC if not skip_attn else 0):
                    tb = wp * 128
                    for g in range(2):
                        # S = q.k^T + bias for heads 6g..6g+5
                        t_s = pss.tile([128, 384], F32)
                        nc.tensor.matmul(t_s[:, :], t_idf,
                                         t_bias[:, 384 * g:384 * g + 384],
                                         start=True, stop=False)
                        for lh in range(6):
                            h = 6 * g + lh
                            hp, mc = h % 2, h // 2
                            lc = mc - 3 * g
                            for w in range(2):
                                nc.tensor.matmul(
                                    t_s[64 * hp:64 * hp + 64,
                                        128 * lc + 64 * w:128 * lc + 64 * w + 64],
                                    t_qk[64 * hp:64 * hp + 64, mc,
                                         tb + 64 * w:tb + 64 * w + 64],
                                    t_qk[64 * hp:64 * hp + 64, 6 + mc,
                                         tb + 64 * w:tb + 64 * w + 64],
                                    start=False, stop=(lh == 5 and w == 1),
                                    tile_position=(64 * hp, 64 * hp))
                        # softmax over m within each (h, w, n) group
                        t_p = ppool.tile([128, 384], BF16)
                        if SKIP_MAX:
                            nc.scalar.activation(out=t_p, in_=t_s[:, :],
                                                 func=AF.Exp, bias=0.0, scale=1.0)
                        else:
                            # exact per-(h,w,n)-group max subtraction
                            t_nm = smpool.tile([128, 6], F32, tag="nm")
                            nc.vector.tensor_reduce(
                                out=t_nm,
                                in_=t_s.rearrange("p (g m) -> p g m", g=6),
                                axis=AX.X, op=ALU.max, negate=True)
                            sv = t_s.rearrange("p (g m) -> p g m", g=6)
                            nc.vector.tensor_add(sv, sv, _bcast_free(t_nm, 64))
                            nc.scalar.activation(out=t_p, in_=t_s[:, :],
                                                 func=AF.Exp, bias=0.0,
                                                 scale=1.0)
                        t_sum = smpool.tile([128, 6], F32, tag="sum")
                        nc.vector.tensor_reduce(
                            out=t_sum, in_=t_p.rearrange("p (g m) -> p g m", g=6),
                            axis=AX.X, op=ALU.add)
                        t_rec = smpool.tile([128, 6], F32, tag="rec")
                        nc.vector.reciprocal(out=t_rec, in_=t_sum)
                        pv = t_p.rearrange("p (g m) -> p g m", g=6)
                        nc.gpsimd.tensor_mul(pv, pv, _bcast_free(t_rec, 64))
                        # P^T: rows (w, m), cols (hp, n)
                        t_t = pst.tile([128, 384], BF16)
                        for b in range(3):
                            nc.tensor.transpose(t_t[:, 128 * b:128 * b + 128],
                                                t_p[:, 128 * b:128 * b + 128], t_id)
                        t_pt = ptpool.tile([128, 384], BF16)
                        nc.vector.tensor_copy(t_pt, t_t)
                        # O = P V token-major; rows (w, n), cols (lh, d)
                        t_O = psO.tile([128, 384], F32, tag="opj")
                        for lh in range(6):
                            h = 6 * g + lh
                            hp, mc = h % 2, h // 2
                            lc = mc - 3 * g
                            for w in range(2):
                                nc.tensor.matmul(
                                    t_O[64 * w:64 * w + 64,
                                        64 * lh:64 * lh + 64],
                                    t_pt[64 * w:64 * w + 64,
                                         128 * lc + 64 * hp:128 * lc + 64 * hp + 64],
                                    t_v[64 * w:64 * w + 64, wp, 64 * h:64 * h + 64],
                                    start=True, stop=True,
                                    tile_position=(64 * w, 64 * w))
                        t_Osb = opool.tile([128, 384], BF16)
                        nc.scalar.activation(out=t_Osb, in_=t_O, func=AF.Identity,
                                             bias=0.0, scale=1.0)
                        # O^T: block b covers heads 6g+2b, 6g+2b+1 -> kc = 3g+b
                        t_ot2 = psot.tile([128, 384], BF16)
                        for b in range(3):
                            nc.tensor.transpose(t_ot2[:, 128 * b:128 * b + 128],
                                                t_Osb[:, 128 * b:128 * b + 128],
                                                t_id)
                        nc.vector.tensor_copy(
                            t_ot[:, 3 * g:3 * g + 3, tb:tb + 128],
                            t_ot2.rearrange("p (a b) -> p a b", a=3))

                # ---- output projection: out^T [pfeat, tok]
                t_out = outpool.tile([128, KC, CHTOK], F32)
                if skip_proj:
                    nc.vector.memset(t_out, 0.0)
                for mc in range(KC if not skip_proj else 0):
                    ps = psO.tile([128, CHTOK], F32, tag="opj")
                    for kc in range(KC):
                        nc.tensor.matmul(
                            ps, t_pw[:, kc, 128 * mc:128 * mc + 128],
                            t_ot[:, kc, :],
                            start=(kc == 0), stop=(kc == KC - 1))
                    nc.scalar.activation(
                        out=t_out[:, mc, :], in_=ps, func=AF.Identity,
                        bias=t_pb[:, mc:mc + 1], scale=1.0)
                nc.sync.dma_start(out=outr[:, :, c0:c0 + CHTOK], in_=t_out)

            loop_ctx = tc.For_i(0, rep, 1) if rep > 1 else contextlib.nullcontext()
            with loop_ctx:
                for ch in range(n_chunk):
                    chunk_body(ch)

    _split_multi_waits(nc)
    return nc


def _get_nc(safe_softmax=False):
    key = ("nc", safe_softmax)
    if key not in _CACHE:
        _CACHE[key] = _build(safe_softmax)
    return _CACHE[key]


def _prep_inputs(x, qkv_w, qkv_b, proj_w, proj_b, rpb_table, rel_pos_index):
    x = np.asarray(x, np.float32)
    qkv_w = np.asarray(qkv_w, np.float32)
    qkv_b = np.asarray(qkv_b, np.float32)
    proj_w = np.asarray(proj_w, np.float32)
    proj_b = np.asarray(proj_b, np.float32)
    rpb_table = np.asarray(rpb_table, np.float32)
    rel_pos_index = np.asarray(rel_pos_index)

    wqk = qkv_w[:, :2 * DIM].copy()
    wqk[:, :DIM] *= SCALE
    wqk_blk = np.ascontiguousarray(
        wqk.reshape(KC, 128, 12, 128).transpose(2, 0, 1, 3))  # [mc, kc, p, m]
    bqk = qkv_b[:2 * DIM].copy()
    bqk[:DIM] *= SCALE
    wv = np.ascontiguousarray(qkv_w[:, 2 * DIM:])
    bv = qkv_b[2 * DIM:]
    pb_eff = proj_b + bv @ proj_w

    # rel-pos bias, gathered and laid out [row=(hp,n), col=(c,w,m)]
    bias_nmh = rpb_table[rel_pos_index]              # [n, m, h]
    bias_dup = np.empty((128, DIM), np.float32)
    for hp in range(2):
        for c in range(6):
            h = 2 * c + hp
            for w in range(2):
                bias_dup[64 * hp:64 * hp + 64,
                         128 * c + 64 * w:128 * c + 64 * w + 64] = bias_nmh[:, :, h]

    xT = np.ascontiguousarray(x.reshape(B * N, DIM).T)      # [768, 65536]
    pdt = (ml_dtypes.bfloat16
           if int(os.environ.get("KERNEL_BF16_PROJ", "1")) else np.float32)
    common = {
        "wqk": np.asarray(wqk_blk.astype(pdt)),
        "wv": np.asarray(wv.astype(pdt)),
        "pw": np.asarray(proj_w.astype(ml_dtypes.bfloat16)),
        "bqk": np.ascontiguousarray(bqk.reshape(12, 128).T),
        "pb": np.ascontiguousarray(pb_eff.reshape(6, 128).T),
        "bias": np.asarray(bias_dup.astype(ml_dtypes.bfloat16)),
        "ident": np.eye(128, dtype=ml_dtypes.bfloat16),
        "identf": np.eye(128, dtype=ml_dtypes.bfloat16),
    }
    in_maps = []
    for c in range(NCORES):
        m = dict(common)
        m["xT"] = np.asarray(np.ascontiguousarray(xT[:, c * TOK:(c + 1) * TOK]).astype(pdt))
        in_maps.append(m)
    return in_maps


def _run(nc, in_maps):
    res = run_bass_kernel_spmd(nc, in_maps, core_ids=list(range(NCORES)))
    out = np.empty((B * N, DIM), np.float32)
    for c in range(NCORES):
        out[c * TOK:(c + 1) * TOK] = res.results[c]["outT"].T
    return out.reshape(B, N, DIM)


def kernel(x, qkv_w, qkv_b, proj_w, proj_b, rpb_table, rel_pos_index):
    in_maps = _prep_inputs(x, qkv_w, qkv_b, proj_w, proj_b,
                           rpb_table, rel_pos_index)
    out = _run(_get_nc(), in_maps)
    if not np.isfinite(out).all():
        # exp overflow/underflow (inputs far outside the reference scale):
        # rerun with the max-subtracted softmax variant
        out = _run(_get_nc(safe_softmax=True), in_maps)
    return out



# revision 19
# speedup vs baseline: 14699.1325x; 14699.1325x over previous
"""Swin-style window attention (B=1024 windows, N=64 tokens, DIM=768, 12 heads)
for 8 Trainium2 NeuronCores.

Strategy: data-parallel over windows (128 windows/core). Everything is
feature-major on device; the host pre-transposes x and post-transposes the
output. Per core:
  - qk^T = (Wqk^T x^T) in fp8e4m3 with DoubleRow perf mode (256-deep
    contraction per pass, ~4x bf16 PE throughput measured). x is fed as a
    TWO-TERM fp8 split (x ~ x8 + x8b, x8b = e4m3 of the residual, which
    lands in e4m3's subnormal range almost exactly) so the x-side
    quantization noise vanishes and only the weight-side fp8 noise
    remains: final rel err ~1.2e-2 vs ~1.8e-2 single-term. fp32 PSUM;
    +bias and the attention scale (q half) ride on the PSUM->SBUF
    activation; fp8 weights are pre-scaled by 64 (undone there too) to
    sit in e4m3's normal range. Output cast fp16.
  - V = x Wv token-major in fp16 (fp16 everywhere over bf16: same PE rate
    here, ~8x lower base quantization error, which buys the fp8 margin)
  - per window-pair: S = q.k^T + rel-pos-bias (PSUM accumulation; bias added
    via identity matmul), softmax along free axis (exp on ACT, grouped sums
    on DVE, normalize on GPSIMD), P^T via PE transposes, O = P V token-major
    (diagonal PE quadrants), O^T via PE transposes (DMA XBAR transposes
    measured slower: HWDGE issue cost dominates), PSUM->SBUF copies split
    across ACT/DVE
  - out^T = proj_w^T O^T + proj_b in fp16, fp32 out, DMA to HBM
Host gathers the 8 outputs and transposes back. Chunk input DMAs are one
3D DMA per stream (HWDGE issue cost is per-instruction).

All matmul quadrant pairs use diagonal tile_position only: concurrent
matmuls with overlapping output partition groups but different row groups
fault the PSUM write port (verified empirically).

The local walrus accepts at most ONE semaphore wait per instruction;
split_multi_waits() hoists extra waits onto same-engine NoOps.
"""
import os
import sys

if "/opt/trn_rl_repo" not in sys.path:
    sys.path.insert(0, "/opt/trn_rl_repo")

import numpy as np
import ml_dtypes

import concourse.bass as bass
import concourse.tile as tile
from concourse import mybir
from concourse.bass_utils import run_bass_kernel_spmd

DIM = 768
HEADS = 12
N = 64            # tokens per window
B = 1024          # windows
NCORES = 8
BC = B // NCORES          # windows per core = 128
TOK = BC * N              # tokens per core = 8192
CHTOK = 512               # tokens per chunk
NCHUNK = TOK // CHTOK     # 16
WPC = CHTOK // 128        # window pairs per chunk = 4
KC = DIM // 128           # 6 contraction chunks
SCALE = (DIM // HEADS) ** -0.5
W8SCALE = 64.0            # fp8 weight pre-scale (undone in activation)

F32 = mybir.dt.float32
F32R = mybir.dt.float32r
BF16 = mybir.dt.bfloat16
FP16 = mybir.dt.float16
F8 = mybir.dt.float8e4
AF = mybir.ActivationFunctionType
ALU = mybir.AluOpType
AX = mybir.AxisListType
DR = mybir.MatmulPerfMode.DoubleRow

_CACHE = {}


def _dt():
    return BF16 if os.environ.get("KERNEL_DT", "fp16") == "bf16" else FP16


def _np_dt():
    return (ml_dtypes.bfloat16 if os.environ.get("KERNEL_DT", "fp16") == "bf16"
            else np.float16)


def _qk_fp8():
    """'0' = none, '1' = q and k, 'k' = k only."""
    return os.environ.get("KERNEL_QK_FP8", "1")


def _split_multi_waits(nc, limit=1):
    """Walrus here encodes at most `limit` sem-waits per instruction; hoist
    extras onto preceding same-engine NoOps (engine streams are in-order)."""
    ctr = 0
    for fn in nc.m.functions:
        for blk in fn.blocks:
            insts = list(blk.instructions)
            out = []
            changed = False
            for inst in insts:
                si = inst.sync_info
                waits = list(si.on_wait) if si is not None else []
                if len(waits) > limit:
                    changed = True
                    extra, keep = waits[:-limit], waits[-limit:]
                    for i in range(0, len(extra), limit):
                        nop = mybir.InstNoOp(name=f"WSPLIT-{ctr}", ins=[], outs=[])
                        ctr += 1
                        nop.engine = inst.engine
                        nop.sync_info = mybir.SyncInfo(
                            on_wait=extra[i:i + limit], on_update=[])
                        nc.register_instruction(nop)
                        out.append(nop)
                    si.on_wait = keep
                out.append(inst)
            if changed:
                while len(blk.instructions):
                    blk.instructions.pop()
                for inst in out:
                    blk.instructions.append(inst)
    return ctr


def _bcast_free(ap, n):
    """AP view broadcasting a [P, G] tile to [P, G, n] via zero-stride."""
    return bass.AP(tensor=ap.tensor, offset=ap.offset,
                   ap=[list(ap.ap[0]), list(ap.ap[1]), [0, n]])


def _build(safe_softmax=False):
    import contextlib

    DT = _dt()
    QK8 = _qk_fp8()
    n8 = {"0": 0, "1": 12, "k": 6}[QK8]   # mc blocks in fp8 (k blocks last)
    nc = bass.Bass()
    d_x = nc.dram_tensor("xT", [DIM, TOK], DT, kind="ExternalInput")
    X8RES = bool(int(os.environ.get("KERNEL_X8_RES", "1")))
    if n8:
        d_x8 = nc.dram_tensor("x8T", [DIM, TOK], F8, kind="ExternalInput")
        if X8RES:
            d_x8b = nc.dram_tensor("x8bT", [DIM, TOK], F8,
                                   kind="ExternalInput")
        d_wqk8 = nc.dram_tensor("wqk8", [n8, KC, 128, 128], F8,
                                kind="ExternalInput")
    if n8 < 12:
        d_wqk = nc.dram_tensor("wqk", [12 - n8, KC, 128, 128], DT,
                               kind="ExternalInput")
    d_wv = nc.dram_tensor("wv", [DIM, DIM], DT, kind="ExternalInput")
    d_pw = nc.dram_tensor("pw", [DIM, DIM], DT, kind="ExternalInput")
    d_bqk = nc.dram_tensor("bqk", [128, 12], F32, kind="ExternalInput")
    d_pb = nc.dram_tensor("pb", [128, 6], F32, kind="ExternalInput")
    d_bias = nc.dram_tensor("bias", [128, DIM], DT, kind="ExternalInput")
    d_id = nc.dram_tensor("ident", [128, 128], DT, kind="ExternalInput")
    d_idf = nc.dram_tensor("identf", [128, 128], DT, kind="ExternalInput")
    d_out = nc.dram_tensor("outT", [DIM, TOK], F32, kind="ExternalOutput")

    xr = d_x.rearrange("(kc p) t -> p kc t", p=128)
    if n8:
        xr8 = d_x8.rearrange("(kc p) t -> p kc t", p=128)
        if X8RES:
            xr8b = d_x8b.rearrange("(kc p) t -> p kc t", p=128)
    wvr = d_wv.rearrange("(kc p) m -> p kc m", p=128)
    pwr = d_pw.rearrange("(kc p) m -> p kc m", p=128)
    outr = d_out.rearrange("(kc p) t -> p kc t", p=128)

    n_chunk = int(os.environ.get("KERNEL_NCHUNK", NCHUNK))
    SKIP_MAX = (not safe_softmax) and bool(int(os.environ.get("KERNEL_SKIP_MAX", "1")))
    OT_DMA = bool(int(os.environ.get("KERNEL_OT_DMA", "0")))
    PT_DMA = bool(int(os.environ.get("KERNEL_PT_DMA", "0")))
    rep = int(os.environ.get("KERNEL_REPEAT", "1"))

    with tile.TileContext(nc) as tc:
        with (
            tc.tile_pool(name="const", bufs=1) as cpool,
            tc.tile_pool(name="xin", bufs=2) as xpool,
            tc.tile_pool(name="qk", bufs=2) as qkpool,
            tc.tile_pool(name="vv", bufs=2) as vpool,
            tc.tile_pool(name="pp", bufs=4) as ppool,
            tc.tile_pool(name="ptp", bufs=4) as ptpool,
            tc.tile_pool(name="osb", bufs=4) as opool,
            tc.tile_pool(name="otc", bufs=2) as otcpool,
            tc.tile_pool(name="outp", bufs=2) as outpool,
            tc.tile_pool(name="smx", bufs=8) as smpool,
            tc.tile_pool(name="psbig", bufs=2, space="PSUM") as psbig,
            tc.tile_pool(name="pss", bufs=2, space="PSUM") as pss,
            tc.tile_pool(name="pst", bufs=1, space="PSUM") as pst,
            tc.tile_pool(name="psO", bufs=2, space="PSUM") as psO,
            tc.tile_pool(name="psot", bufs=1, space="PSUM") as psot,
        ):
            t_wqk8 = (cpool.tile([128, n8, KC, 128], F8, name="t_wqk8")
                      if n8 else None)
            t_wqkd = (cpool.tile([128, 12 - n8, KC, 128], DT, name="t_wqkd")
                      if n8 < 12 else None)
            t_wv = cpool.tile([128, KC, DIM], DT)
            t_pw = cpool.tile([128, KC, DIM], DT)
            t_bqk = cpool.tile([128, 12], F32)
            t_pb = cpool.tile([128, 6], F32)
            t_bias = cpool.tile([128, DIM], DT)
            t_id = cpool.tile([128, 128], DT)
            t_idf = cpool.tile([128, 128], DT)
            # smalls first, then per-mc weight blocks so the first matmul
            # group unblocks after ~0.8MB instead of the full weight load
            nc.sync.dma_start(out=t_bqk, in_=d_bqk[:, :])
            nc.sync.dma_start(out=t_bias, in_=d_bias[:, :])
            nc.sync.dma_start(out=t_id, in_=d_id[:, :])
            nc.sync.dma_start(out=t_idf, in_=d_idf[:, :])
            nc.sync.dma_start(out=t_pb, in_=d_pb[:, :])
            # weight tile views indexed by global mc: fp8 blocks live at the
            # END (k half in 'k' mode; everything in '1' mode)
            def wtile(mc):
                if mc >= 12 - n8:
                    return t_wqk8, mc - (12 - n8)
                return t_wqkd, mc

            wqk2d = (d_wqk.rearrange("mc kc p m -> p mc kc m")
                     if n8 < 12 else None)
            wqk28 = (d_wqk8.rearrange("mc kc p m -> p mc kc m")
                     if n8 else None)

            def wload(mc):
                t, i = wtile(mc)
                src = wqk28 if t is t_wqk8 else wqk2d
                nc.sync.dma_start(out=t[:, i, :, :], in_=src[:, i, :, :])

            # first mc block, then chunk-0 x, then the rest of the weights:
            # the first projection group unblocks after ~1MB of DMA
            wload(0)
            t_x0 = xpool.tile([128, KC, CHTOK], DT)
            if n8:
                t_x80 = xpool.tile([128, KC, CHTOK], F8, tag="x8")
                nc.sync.dma_start(out=t_x80, in_=xr8[:, :, 0:CHTOK])
                if X8RES:
                    t_x8b0 = xpool.tile([128, KC, CHTOK], F8, tag="x8b")
                    nc.sync.dma_start(out=t_x8b0, in_=xr8b[:, :, 0:CHTOK])
            nc.sync.dma_start(out=t_x0, in_=xr[:, :, 0:CHTOK])
            for mc in range(1, 12):
                wload(mc)
            for kc in range(KC):
                nc.sync.dma_start(out=t_wv[:, kc, :], in_=wvr[:, kc, :])
            for kc in range(KC):
                nc.sync.dma_start(out=t_pw[:, kc, :], in_=pwr[:, kc, :])

            skip_qkv = bool(int(os.environ.get("KERNEL_SKIP_QKV", "0")))
            skip_attn = bool(int(os.environ.get("KERNEL_SKIP_ATTN", "0")))
            skip_proj = bool(int(os.environ.get("KERNEL_SKIP_PROJ", "0")))

            def chunk_body(ch):
                c0 = ch * CHTOK
                if ch == 0:
                    t_x = t_x0
                    t_x8 = t_x80 if n8 else None
                    t_x8b = t_x8b0 if (n8 and X8RES) else None
                else:
                    t_x = xpool.tile([128, KC, CHTOK], DT)
                    if n8:
                        t_x8 = xpool.tile([128, KC, CHTOK], F8, tag="x8")
                        nc.sync.dma_start(out=t_x8,
                                          in_=xr8[:, :, c0:c0 + CHTOK])
                        if X8RES:
                            t_x8b = xpool.tile([128, KC, CHTOK], F8, tag="x8b")
                            nc.sync.dma_start(out=t_x8b,
                                              in_=xr8b[:, :, c0:c0 + CHTOK])
                    nc.sync.dma_start(out=t_x,
                                      in_=xr[:, :, c0:c0 + CHTOK])

                # ---- q/k projection: qk^T [feat, tok] -> DT
                # fp8 DoubleRow: 3 passes of 256-deep contraction, 2x rate.
                # The attn scale rides on the q activations (mc<6); the fp8
                # weight pre-scale is undone the same way.
                t_qk = qkpool.tile([128, 12, CHTOK], DT)
                if skip_qkv:
                    nc.vector.memset(t_qk, 0.0)
                for mc in range(12 if not skip_qkv else 0):
                    t_w, i = wtile(mc)
                    ps = psbig.tile([128, CHTOK], F32, tag="big")
                    if t_w is t_wqk8:
                        # x ~ x8 + x8b (fp8 two-term split): both terms hit
                        # the same fp8 weights in one PSUM accumulation
                        srcs = [t_x8, t_x8b] if X8RES else [t_x8]
                        for j in range(3):
                            for si, t_xs in enumerate(srcs):
                                nc.tensor.matmul(
                                    ps, t_w[:, i, 2 * j:2 * j + 2, :],
                                    t_xs[:, 2 * j:2 * j + 2, :],
                                    start=(si == 0 and j == 0),
                                    stop=(si == len(srcs) - 1 and j == 2),
                                    perf_mode=DR)
                        sc = 1.0 / W8SCALE
                    else:
                        for kc in range(KC):
                            nc.tensor.matmul(
                                ps, t_w[:, i, kc, :],
                                t_x[:, kc, :],
                                start=(kc == 0), stop=(kc == KC - 1))
                        sc = 1.0
                    if mc < 6:
                        sc *= SCALE
                    nc.scalar.activation(
                        out=t_qk[:, mc, :], in_=ps, func=AF.Identity,
                        bias=t_bqk[:, mc:mc + 1], scale=sc)

                # ---- V projection: token-major [tok, feat] -> fp16
                t_v = vpool.tile([128, WPC, DIM], DT)
                if skip_qkv:
                    nc.vector.memset(t_v, 0.0)
                for tch in range(WPC if not skip_qkv else 0):
                    for half in range(2):
                        n0 = 384 * half
                        ps = psbig.tile([128, 384], F32, tag="big")
                        for kc in range(KC):
                            nc.tensor.matmul(
                                ps, t_x[:, kc, 128 * tch:128 * tch + 128],
                                t_wv[:, kc, n0:n0 + 384],
                                start=(kc == 0), stop=(kc == KC - 1))
                        nc.vector.tensor_copy(t_v[:, tch, n0:n0 + 384], ps)

                # ---- attention per window pair, split into half-head
                # sub-chains (heads 6g..6g+5) so S/O/T are 1 PSUM bank
                # each and S/O double-buffer: deep cross-chain pipelining.
                t_ot = otcpool.tile([128, KC, CHTOK], DT)
                if skip_attn:
                    nc.vector.memset(t_ot, 0.0)
                for wp in range(WPC if not skip_attn else 0):
                    tb = wp * 128
                    for g in range(2):
                        # S = q.k^T + bias for heads 6g..6g+5
                        t_s = pss.tile([128, 384], F32)
                        nc.tensor.matmul(t_s[:, :], t_idf,
                                         t_bias[:, 384 * g:384 * g + 384],
                                         start=True, stop=False)
                        for lh in range(6):
                            h = 6 * g + lh
                            hp, mc = h % 2, h // 2
                            lc = mc - 3 * g
                            for w in range(2):
                                nc.tensor.matmul(
                                    t_s[64 * hp:64 * hp + 64,
                                        128 * lc + 64 * w:128 * lc + 64 * w + 64],
                                    t_qk[64 * hp:64 * hp + 64, mc,
                                         tb + 64 * w:tb + 64 * w + 64],
                                    t_qk[64 * hp:64 * hp + 64, 6 + mc,
                                         tb + 64 * w:tb + 64 * w + 64],
                                    start=False, stop=(lh == 5 and w == 1),
                                    tile_position=(64 * hp, 64 * hp))
                        # softmax over m within each (h, w, n) group
                        t_p = ppool.tile([128, 384], DT)
                        if SKIP_MAX:
                            nc.scalar.activation(out=t_p, in_=t_s[:, :],
                                                 func=AF.Exp, bias=0.0,
                                                 scale=1.0)
                        else:
                            # exact per-(h,w,n)-group max subtraction
                            t_nm = smpool.tile([128, 6], F32, tag="nm")
                            nc.vector.tensor_reduce(
                                out=t_nm,
                                in_=t_s.rearrange("p (g m) -> p g m", g=6),
                                axis=AX.X, op=ALU.max, negate=True)
                            sv = t_s.rearrange("p (g m) -> p g m", g=6)
                            nc.vector.tensor_add(sv, sv, _bcast_free(t_nm, 64))
                            nc.scalar.activation(out=t_p, in_=t_s[:, :],
                                                 func=AF.Exp, bias=0.0,
                                                 scale=1.0)
                        t_sum = smpool.tile([128, 6], F32, tag="sum")
                        nc.vector.tensor_reduce(
                            out=t_sum, in_=t_p.rearrange("p (g m) -> p g m", g=6),
                            axis=AX.X, op=ALU.add)
                        t_rec = smpool.tile([128, 6], F32, tag="rec")
                        nc.vector.reciprocal(out=t_rec, in_=t_sum)
                        pv = t_p.rearrange("p (g m) -> p g m", g=6)
                        nc.gpsimd.tensor_mul(pv, pv, _bcast_free(t_rec, 64))
                        # P^T: rows (w, m), cols (hp, n)
                        t_pt = ptpool.tile([128, 384], DT)
                        if PT_DMA:
                            for b in range(3):
                                nc.sync.dma_start(
                                    out=t_pt[:, 128 * b:128 * b + 128],
                                    in_=t_p[:, 128 * b:128 * b + 128],
                                    transpose=True)
                        else:
                            t_t = pst.tile([128, 384], DT)
                            for b in range(3):
                                nc.tensor.transpose(
                                    t_t[:, 128 * b:128 * b + 128],
                                    t_p[:, 128 * b:128 * b + 128], t_id)
                            nc.vector.tensor_copy(t_pt, t_t)
                        # O = P V token-major; rows (w, n), cols (lh, d)
                        t_O = psO.tile([128, 384], F32, tag="opj")
                        for lh in range(6):
                            h = 6 * g + lh
                            hp, mc = h % 2, h // 2
                            lc = mc - 3 * g
                            for w in range(2):
                                nc.tensor.matmul(
                                    t_O[64 * w:64 * w + 64,
                                        64 * lh:64 * lh + 64],
                                    t_pt[64 * w:64 * w + 64,
                                         128 * lc + 64 * hp:128 * lc + 64 * hp + 64],
                                    t_v[64 * w:64 * w + 64, wp, 64 * h:64 * h + 64],
                                    start=True, stop=True,
                                    tile_position=(64 * w, 64 * w))
                        t_Osb = opool.tile([128, 384], DT)
                        nc.vector.tensor_copy(t_Osb, t_O)
                        # O^T: block b covers heads 6g+2b, 6g+2b+1 -> kc = 3g+b
                        if OT_DMA:
                            for b in range(3):
                                nc.sync.dma_start(
                                    out=t_ot[:, 3 * g + b, tb:tb + 128],
                                    in_=t_Osb[:, 128 * b:128 * b + 128],
                                    transpose=True)
                        else:
                            t_ot2 = psot.tile([128, 384], DT)
                            for b in range(3):
                                nc.tensor.transpose(
                                    t_ot2[:, 128 * b:128 * b + 128],
                                    t_Osb[:, 128 * b:128 * b + 128],
                                    t_id)
                            nc.vector.tensor_copy(
                                t_ot[:, 3 * g:3 * g + 3, tb:tb + 128],
                                t_ot2.rearrange("p (a b) -> p a b", a=3))

                # ---- output projection: out^T [pfeat, tok]
                t_out = outpool.tile([128, KC, CHTOK], F32)
                if skip_proj:
                    nc.vector.memset(t_out, 0.0)
                for mc in range(KC if not skip_proj else 0):
                    ps = psO.tile([128, CHTOK], F32, tag="opj")
                    for kc in range(KC):
                        nc.tensor.matmul(
                            ps, t_pw[:, kc, 128 * mc:128 * mc + 128],
                            t_ot[:, kc, :],
                            start=(kc == 0), stop=(kc == KC - 1))
                    nc.scalar.activation(
                        out=t_out[:, mc, :], in_=ps, func=AF.Identity,
                        bias=t_pb[:, mc:mc + 1], scale=1.0)
                nc.sync.dma_start(out=outr[:, :, c0:c0 + CHTOK], in_=t_out)

            loop_ctx = tc.For_i(0, rep, 1) if rep > 1 else contextlib.nullcontext()
            with loop_ctx:
                for ch in range(n_chunk):
                    chunk_body(ch)

    _split_multi_waits(nc)
    return nc


def _get_nc(safe_softmax=False):
    key = ("nc", safe_softmax, os.environ.get("KERNEL_DT", "fp16"),
           _qk_fp8(), os.environ.get("KERNEL_X8_RES", "1"))
    if key not in _CACHE:
        _CACHE[key] = _build(safe_softmax)
    return _CACHE[key]


def _prep_inputs(x, qkv_w, qkv_b, proj_w, proj_b, rpb_table, rel_pos_index):
    x = np.asarray(x, np.float32)
    qkv_w = np.asarray(qkv_w, np.float32)
    qkv_b = np.asarray(qkv_b, np.float32)
    proj_w = np.asarray(proj_w, np.float32)
    proj_b = np.asarray(proj_b, np.float32)
    rpb_table = np.asarray(rpb_table, np.float32)
    rel_pos_index = np.asarray(rel_pos_index)

    np_dt = _np_dt()
    QK8 = _qk_fp8()
    n8 = {"0": 0, "1": 12, "k": 6}[QK8]

    wqk = qkv_w[:, :2 * DIM].copy()
    # the attn scale is applied on the q activations device-side; the bias
    # is added after that scale, so fold it into the bias here
    bqk = qkv_b[:2 * DIM].copy()
    bqk[:DIM] *= SCALE
    wv = np.ascontiguousarray(qkv_w[:, 2 * DIM:])
    bv = qkv_b[2 * DIM:]
    pb_eff = proj_b + bv @ proj_w

    # rel-pos bias, gathered and laid out [row=(hp,n), col=(c,w,m)]
    bias_nmh = rpb_table[rel_pos_index]              # [n, m, h]
    bias_dup = np.empty((128, DIM), np.float32)
    for hp in range(2):
        for c in range(6):
            h = 2 * c + hp
            for w in range(2):
                bias_dup[64 * hp:64 * hp + 64,
                         128 * c + 64 * w:128 * c + 64 * w + 64] = bias_nmh[:, :, h]

    xT = np.ascontiguousarray(x.reshape(B * N, DIM).T)      # [768, 65536]
    wqk_blk = np.ascontiguousarray(
        wqk.reshape(KC, 128, 12, 128).transpose(2, 0, 1, 3))  # [mc, kc, p, m]
    common = {
        "wv": np.asarray(wv.astype(np_dt)),
        "pw": np.asarray(proj_w.astype(np_dt)),
        "bqk": np.ascontiguousarray(bqk.reshape(12, 128).T),
        "pb": np.ascontiguousarray(pb_eff.reshape(6, 128).T),
        "bias": np.asarray(bias_dup.astype(np_dt)),
        "ident": np.eye(128, dtype=np_dt),
        "identf": np.eye(128, dtype=np_dt),
    }
    X8RES = bool(int(os.environ.get("KERNEL_X8_RES", "1")))
    if n8:
        common["wqk8"] = np.asarray(
            (wqk_blk[12 - n8:] * W8SCALE).astype(ml_dtypes.float8_e4m3))
        x8T = np.asarray(xT.astype(ml_dtypes.float8_e4m3))
        if X8RES:
            x8bT = np.asarray(
                (xT - x8T.astype(np.float32)).astype(ml_dtypes.float8_e4m3))
    if n8 < 12:
        common["wqk"] = np.asarray(wqk_blk[:12 - n8].astype(np_dt))
    xTd = np.asarray(xT.astype(np_dt))
    in_maps = []
    for c in range(NCORES):
        m = dict(common)
        m["xT"] = np.ascontiguousarray(xTd[:, c * TOK:(c + 1) * TOK])
        if n8:
            m["x8T"] = np.ascontiguousarray(x8T[:, c * TOK:(c + 1) * TOK])
            if X8RES:
                m["x8bT"] = np.ascontiguousarray(
                    x8bT[:, c * TOK:(c + 1) * TOK])
        in_maps.append(m)
    return in_maps


def _run(nc, in_maps):
    res = run_bass_kernel_spmd(nc, in_maps, core_ids=list(range(NCORES)))
    out = np.empty((B * N, DIM), np.float32)
    for c in range(NCORES):
        out[c * TOK:(c + 1) * TOK] = res.results[c]["outT"].T
    return out.reshape(B, N, DIM)


def kernel(x, qkv_w, qkv_b, proj_w, proj_b, rpb_table, rel_pos_index):
    in_maps = _prep_inputs(x, qkv_w, qkv_b, proj_w, proj_b,
                           rpb_table, rel_pos_index)
    out = _run(_get_nc(), in_maps)
    if not np.isfinite(out).all():
        # exp overflow/underflow (inputs far outside the reference scale):
        # rerun with the max-subtracted softmax variant
        out = _run(_get_nc(safe_softmax=True), in_maps)
    return out


# revision 20
# speedup vs baseline: 14823.8248x; 1.0085x over previous
"""Swin-style window attention (B=1024 windows, N=64 tokens, DIM=768, 12 heads)
for 8 Trainium2 NeuronCores.

Strategy: data-parallel over windows (128 windows/core). Everything is
feature-major on device; the host pre-transposes x and post-transposes the
output. Per core:
  - qk^T = (Wqk^T x^T) in fp8e4m3 with DoubleRow perf mode (256-deep
    contraction per pass, ~4x bf16 PE throughput measured). x is fed as a
    TWO-TERM fp8 split (x ~ x8 + x8b, x8b = e4m3 of the residual, which
    lands in e4m3's subnormal range almost exactly) so the x-side
    quantization noise vanishes and only the weight-side fp8 noise
    remains: final rel err ~1.2e-2 vs ~1.8e-2 single-term. fp32 PSUM;
    +bias and the attention scale (q half) ride on the PSUM->SBUF
    activation; fp8 weights are pre-scaled by 64 (undone there too) to
    sit in e4m3's normal range. Output cast fp16.
  - V = x Wv token-major in fp16 (fp16 everywhere over bf16: same PE rate
    here, ~8x lower base quantization error, which buys the fp8 margin)
  - per window-pair: S = q.k^T + rel-pos-bias (PSUM accumulation; bias added
    via identity matmul), softmax along free axis (exp on ACT, grouped sums
    on DVE, normalize on GPSIMD), P^T via PE transposes, O = P V token-major
    (diagonal PE quadrants), O^T via PE transposes (DMA XBAR transposes
    measured slower: HWDGE issue cost dominates), PSUM->SBUF copies split
    across ACT/DVE
  - out^T = proj_w^T O^T + proj_b in fp16, fp32 out, DMA to HBM
Host gathers the 8 outputs and transposes back. Chunk input DMAs are one
3D DMA per stream (HWDGE issue cost is per-instruction).

All matmul quadrant pairs use diagonal tile_position only: concurrent
matmuls with overlapping output partition groups but different row groups
fault the PSUM write port (verified empirically).

The local walrus accepts at most ONE semaphore wait per instruction;
split_multi_waits() hoists extra waits onto same-engine NoOps.
"""
import os
import sys

if "/opt/trn_rl_repo" not in sys.path:
    sys.path.insert(0, "/opt/trn_rl_repo")

import numpy as np
import ml_dtypes

import concourse.bass as bass
import concourse.tile as tile
from concourse import mybir
from concourse.bass_utils import run_bass_kernel_spmd

DIM = 768
HEADS = 12
N = 64            # tokens per window
B = 1024          # windows
NCORES = 8
BC = B // NCORES          # windows per core = 128
TOK = BC * N              # tokens per core = 8192
CHTOK = 512               # tokens per chunk
NCHUNK = TOK // CHTOK     # 16
WPC = CHTOK // 128        # window pairs per chunk = 4
KC = DIM // 128           # 6 contraction chunks
SCALE = (DIM // HEADS) ** -0.5
W8SCALE = 64.0            # fp8 weight pre-scale (undone in activation)

F32 = mybir.dt.float32
F32R = mybir.dt.float32r
BF16 = mybir.dt.bfloat16
FP16 = mybir.dt.float16
F8 = mybir.dt.float8e4
AF = mybir.ActivationFunctionType
ALU = mybir.AluOpType
AX = mybir.AxisListType
DR = mybir.MatmulPerfMode.DoubleRow

_CACHE = {}


def _dt():
    return BF16 if os.environ.get("KERNEL_DT", "fp16") == "bf16" else FP16


def _np_dt():
    return (ml_dtypes.bfloat16 if os.environ.get("KERNEL_DT", "fp16") == "bf16"
            else np.float16)


def _qk_fp8():
    """'0' = none, '1' = q and k, 'k' = k only."""
    return os.environ.get("KERNEL_QK_FP8", "1")


def _split_multi_waits(nc, limit=1):
    """Walrus here encodes at most `limit` sem-waits per instruction; hoist
    extras onto preceding same-engine NoOps (engine streams are in-order)."""
    ctr = 0
    for fn in nc.m.functions:
        for blk in fn.blocks:
            insts = list(blk.instructions)
            out = []
            changed = False
            for inst in insts:
                si = inst.sync_info
                waits = list(si.on_wait) if si is not None else []
                if len(waits) > limit:
                    changed = True
                    extra, keep = waits[:-limit], waits[-limit:]
                    for i in range(0, len(extra), limit):
                        nop = mybir.InstNoOp(name=f"WSPLIT-{ctr}", ins=[], outs=[])
                        ctr += 1
                        nop.engine = inst.engine
                        nop.sync_info = mybir.SyncInfo(
                            on_wait=extra[i:i + limit], on_update=[])
                        nc.register_instruction(nop)
                        out.append(nop)
                    si.on_wait = keep
                out.append(inst)
            if changed:
                while len(blk.instructions):
                    blk.instructions.pop()
                for inst in out:
                    blk.instructions.append(inst)
    return ctr


def _bcast_free(ap, n):
    """AP view broadcasting a [P, G] tile to [P, G, n] via zero-stride."""
    return bass.AP(tensor=ap.tensor, offset=ap.offset,
                   ap=[list(ap.ap[0]), list(ap.ap[1]), [0, n]])


def _build(safe_softmax=False):
    import contextlib

    DT = _dt()
    QK8 = _qk_fp8()
    n8 = {"0": 0, "1": 12, "k": 6}[QK8]   # mc blocks in fp8 (k blocks last)
    nc = bass.Bass()
    d_x = nc.dram_tensor("xT", [DIM, TOK], DT, kind="ExternalInput")
    X8RES = bool(int(os.environ.get("KERNEL_X8_RES", "1")))
    if n8:
        d_x8 = nc.dram_tensor("x8T", [DIM, TOK], F8, kind="ExternalInput")
        if X8RES:
            d_x8b = nc.dram_tensor("x8bT", [DIM, TOK], F8,
                                   kind="ExternalInput")
        d_wqk8 = nc.dram_tensor("wqk8", [n8, KC, 128, 128], F8,
                                kind="ExternalInput")
    if n8 < 12:
        d_wqk = nc.dram_tensor("wqk", [12 - n8, KC, 128, 128], DT,
                               kind="ExternalInput")
    d_wv = nc.dram_tensor("wv", [DIM, DIM], DT, kind="ExternalInput")
    d_pw = nc.dram_tensor("pw", [DIM, DIM], DT, kind="ExternalInput")
    d_bqk = nc.dram_tensor("bqk", [128, 12], F32, kind="ExternalInput")
    d_pb = nc.dram_tensor("pb", [128, 6], F32, kind="ExternalInput")
    d_bias = nc.dram_tensor("bias", [128, DIM], DT, kind="ExternalInput")
    d_id = nc.dram_tensor("ident", [128, 128], DT, kind="ExternalInput")
    d_idf = nc.dram_tensor("identf", [128, 128], DT, kind="ExternalInput")
    d_out = nc.dram_tensor("outT", [DIM, TOK], DT, kind="ExternalOutput")

    xr = d_x.rearrange("(kc p) t -> p kc t", p=128)
    if n8:
        xr8 = d_x8.rearrange("(kc p) t -> p kc t", p=128)
        if X8RES:
            xr8b = d_x8b.rearrange("(kc p) t -> p kc t", p=128)
    wvr = d_wv.rearrange("(kc p) m -> p kc m", p=128)
    pwr = d_pw.rearrange("(kc p) m -> p kc m", p=128)
    outr = d_out.rearrange("(kc p) t -> p kc t", p=128)

    n_chunk = int(os.environ.get("KERNEL_NCHUNK", NCHUNK))
    SKIP_MAX = (not safe_softmax) and bool(int(os.environ.get("KERNEL_SKIP_MAX", "1")))
    OT_DMA = bool(int(os.environ.get("KERNEL_OT_DMA", "0")))
    PT_DMA = bool(int(os.environ.get("KERNEL_PT_DMA", "0")))
    rep = int(os.environ.get("KERNEL_REPEAT", "1"))

    with tile.TileContext(nc) as tc:
        with (
            tc.tile_pool(name="const", bufs=1) as cpool,
            tc.tile_pool(name="xin", bufs=2) as xpool,
            tc.tile_pool(name="qk", bufs=2) as qkpool,
            tc.tile_pool(name="vv", bufs=2) as vpool,
            tc.tile_pool(name="pp", bufs=4) as ppool,
            tc.tile_pool(name="ptp", bufs=4) as ptpool,
            tc.tile_pool(name="osb", bufs=4) as opool,
            tc.tile_pool(name="otc", bufs=2) as otcpool,
            tc.tile_pool(name="outp", bufs=2) as outpool,
            tc.tile_pool(name="smx", bufs=8) as smpool,
            tc.tile_pool(name="psbig", bufs=2, space="PSUM") as psbig,
            tc.tile_pool(name="pss", bufs=2, space="PSUM") as pss,
            tc.tile_pool(name="pst", bufs=1, space="PSUM") as pst,
            tc.tile_pool(name="psO", bufs=2, space="PSUM") as psO,
            tc.tile_pool(name="psot", bufs=1, space="PSUM") as psot,
        ):
            t_wqk8 = (cpool.tile([128, n8, KC, 128], F8, name="t_wqk8")
                      if n8 else None)
            t_wqkd = (cpool.tile([128, 12 - n8, KC, 128], DT, name="t_wqkd")
                      if n8 < 12 else None)
            t_wv = cpool.tile([128, KC, DIM], DT)
            t_pw = cpool.tile([128, KC, DIM], DT)
            t_bqk = cpool.tile([128, 12], F32)
            t_pb = cpool.tile([128, 6], F32)
            t_bias = cpool.tile([128, DIM], DT)
            t_id = cpool.tile([128, 128], DT)
            t_idf = cpool.tile([128, 128], DT)
            # smalls first, then per-mc weight blocks so the first matmul
            # group unblocks after ~0.8MB instead of the full weight load
            nc.sync.dma_start(out=t_bqk, in_=d_bqk[:, :])
            nc.sync.dma_start(out=t_bias, in_=d_bias[:, :])
            nc.sync.dma_start(out=t_id, in_=d_id[:, :])
            nc.sync.dma_start(out=t_idf, in_=d_idf[:, :])
            nc.sync.dma_start(out=t_pb, in_=d_pb[:, :])
            # weight tile views indexed by global mc: fp8 blocks live at the
            # END (k half in 'k' mode; everything in '1' mode)
            def wtile(mc):
                if mc >= 12 - n8:
                    return t_wqk8, mc - (12 - n8)
                return t_wqkd, mc

            wqk2d = (d_wqk.rearrange("mc kc p m -> p mc kc m")
                     if n8 < 12 else None)
            wqk28 = (d_wqk8.rearrange("mc kc p m -> p mc kc m")
                     if n8 else None)

            def wload(mc):
                t, i = wtile(mc)
                src = wqk28 if t is t_wqk8 else wqk2d
                nc.sync.dma_start(out=t[:, i, :, :], in_=src[:, i, :, :])

            # first mc block, then chunk-0 x, then the rest of the weights:
            # the first projection group unblocks after ~1MB of DMA
            wload(0)
            t_x0 = xpool.tile([128, KC, CHTOK], DT)
            if n8:
                t_x80 = xpool.tile([128, KC, CHTOK], F8, tag="x8")
                nc.sync.dma_start(out=t_x80, in_=xr8[:, :, 0:CHTOK])
                if X8RES:
                    t_x8b0 = xpool.tile([128, KC, CHTOK], F8, tag="x8b")
                    nc.sync.dma_start(out=t_x8b0, in_=xr8b[:, :, 0:CHTOK])
            nc.sync.dma_start(out=t_x0, in_=xr[:, :, 0:CHTOK])
            for mc in range(1, 12):
                wload(mc)
            for kc in range(KC):
                nc.sync.dma_start(out=t_wv[:, kc, :], in_=wvr[:, kc, :])
            for kc in range(KC):
                nc.sync.dma_start(out=t_pw[:, kc, :], in_=pwr[:, kc, :])

            skip_qkv = bool(int(os.environ.get("KERNEL_SKIP_QKV", "0")))
            skip_attn = bool(int(os.environ.get("KERNEL_SKIP_ATTN", "0")))
            skip_proj = bool(int(os.environ.get("KERNEL_SKIP_PROJ", "0")))

            def chunk_body(ch):
                c0 = ch * CHTOK
                if ch == 0:
                    t_x = t_x0
                    t_x8 = t_x80 if n8 else None
                    t_x8b = t_x8b0 if (n8 and X8RES) else None
                else:
                    t_x = xpool.tile([128, KC, CHTOK], DT)
                    if n8:
                        t_x8 = xpool.tile([128, KC, CHTOK], F8, tag="x8")
                        nc.sync.dma_start(out=t_x8,
                                          in_=xr8[:, :, c0:c0 + CHTOK])
                        if X8RES:
                            t_x8b = xpool.tile([128, KC, CHTOK], F8, tag="x8b")
                            nc.sync.dma_start(out=t_x8b,
                                              in_=xr8b[:, :, c0:c0 + CHTOK])
                    nc.sync.dma_start(out=t_x,
                                      in_=xr[:, :, c0:c0 + CHTOK])

                # ---- q/k projection: qk^T [feat, tok] -> DT
                # fp8 DoubleRow: 3 passes of 256-deep contraction, 2x rate.
                # The attn scale rides on the q activations (mc<6); the fp8
                # weight pre-scale is undone the same way.
                t_qk = qkpool.tile([128, 12, CHTOK], DT)
                if skip_qkv:
                    nc.vector.memset(t_qk, 0.0)
                for mc in range(12 if not skip_qkv else 0):
                    t_w, i = wtile(mc)
                    ps = psbig.tile([128, CHTOK], F32, tag="big")
                    if t_w is t_wqk8:
                        # x ~ x8 + x8b (fp8 two-term split): both terms hit
                        # the same fp8 weights in one PSUM accumulation
                        srcs = [t_x8, t_x8b] if X8RES else [t_x8]
                        for j in range(3):
                            for si, t_xs in enumerate(srcs):
                                nc.tensor.matmul(
                                    ps, t_w[:, i, 2 * j:2 * j + 2, :],
                                    t_xs[:, 2 * j:2 * j + 2, :],
                                    start=(si == 0 and j == 0),
                                    stop=(si == len(srcs) - 1 and j == 2),
                                    perf_mode=DR)
                        sc = 1.0 / W8SCALE
                    else:
                        for kc in range(KC):
                            nc.tensor.matmul(
                                ps, t_w[:, i, kc, :],
                                t_x[:, kc, :],
                                start=(kc == 0), stop=(kc == KC - 1))
                        sc = 1.0
                    if mc < 6:
                        sc *= SCALE
                    nc.scalar.activation(
                        out=t_qk[:, mc, :], in_=ps, func=AF.Identity,
                        bias=t_bqk[:, mc:mc + 1], scale=sc)

                # ---- V projection: token-major [tok, feat] -> fp16
                t_v = vpool.tile([128, WPC, DIM], DT)
                if skip_qkv:
                    nc.vector.memset(t_v, 0.0)
                for tch in range(WPC if not skip_qkv else 0):
                    for half in range(2):
                        n0 = 384 * half
                        ps = psbig.tile([128, 384], F32, tag="big")
                        for kc in range(KC):
                            nc.tensor.matmul(
                                ps, t_x[:, kc, 128 * tch:128 * tch + 128],
                                t_wv[:, kc, n0:n0 + 384],
                                start=(kc == 0), stop=(kc == KC - 1))
                        nc.vector.tensor_copy(t_v[:, tch, n0:n0 + 384], ps)

                # ---- attention per window pair, split into half-head
                # sub-chains (heads 6g..6g+5) so S/O/T are 1 PSUM bank
                # each and S/O double-buffer: deep cross-chain pipelining.
                t_ot = otcpool.tile([128, KC, CHTOK], DT)
                if skip_attn:
                    nc.vector.memset(t_ot, 0.0)
                for wp in range(WPC if not skip_attn else 0):
                    tb = wp * 128
                    for g in range(2):
                        # S = q.k^T + bias for heads 6g..6g+5
                        t_s = pss.tile([128, 384], F32)
                        nc.tensor.matmul(t_s[:, :], t_idf,
                                         t_bias[:, 384 * g:384 * g + 384],
                                         start=True, stop=False)
                        for lh in range(6):
                            h = 6 * g + lh
                            hp, mc = h % 2, h // 2
                            lc = mc - 3 * g
                            for w in range(2):
                                nc.tensor.matmul(
                                    t_s[64 * hp:64 * hp + 64,
                                        128 * lc + 64 * w:128 * lc + 64 * w + 64],
                                    t_qk[64 * hp:64 * hp + 64, mc,
                                         tb + 64 * w:tb + 64 * w + 64],
                                    t_qk[64 * hp:64 * hp + 64, 6 + mc,
                                         tb + 64 * w:tb + 64 * w + 64],
                                    start=False, stop=(lh == 5 and w == 1),
                                    tile_position=(64 * hp, 64 * hp))
                        # softmax over m within each (h, w, n) group
                        t_p = ppool.tile([128, 384], DT)
                        if SKIP_MAX:
                            nc.scalar.activation(out=t_p, in_=t_s[:, :],
                                                 func=AF.Exp, bias=0.0,
                                                 scale=1.0)
                        else:
                            # exact per-(h,w,n)-group max subtraction
                            t_nm = smpool.tile([128, 6], F32, tag="nm")
                            nc.vector.tensor_reduce(
                                out=t_nm,
                                in_=t_s.rearrange("p (g m) -> p g m", g=6),
                                axis=AX.X, op=ALU.max, negate=True)
                            sv = t_s.rearrange("p (g m) -> p g m", g=6)
                            nc.vector.tensor_add(sv, sv, _bcast_free(t_nm, 64))
                            nc.scalar.activation(out=t_p, in_=t_s[:, :],
                                                 func=AF.Exp, bias=0.0,
                                                 scale=1.0)
                        t_sum = smpool.tile([128, 6], F32, tag="sum")
                        nc.vector.tensor_reduce(
                            out=t_sum, in_=t_p.rearrange("p (g m) -> p g m", g=6),
                            axis=AX.X, op=ALU.add)
                        t_rec = smpool.tile([128, 6], F32, tag="rec")
                        nc.vector.reciprocal(out=t_rec, in_=t_sum)
                        pv = t_p.rearrange("p (g m) -> p g m", g=6)
                        nc.gpsimd.tensor_mul(pv, pv, _bcast_free(t_rec, 64))
                        # P^T: rows (w, m), cols (hp, n)
                        t_pt = ptpool.tile([128, 384], DT)
                        if PT_DMA:
                            for b in range(3):
                                nc.sync.dma_start(
                                    out=t_pt[:, 128 * b:128 * b + 128],
                                    in_=t_p[:, 128 * b:128 * b + 128],
                                    transpose=True)
                        else:
                            t_t = pst.tile([128, 384], DT)
                            for b in range(3):
                                nc.tensor.transpose(
                                    t_t[:, 128 * b:128 * b + 128],
                                    t_p[:, 128 * b:128 * b + 128], t_id)
                            nc.vector.tensor_copy(t_pt, t_t)
                        # O = P V token-major; rows (w, n), cols (lh, d)
                        t_O = psO.tile([128, 384], F32, tag="opj")
                        for lh in range(6):
                            h = 6 * g + lh
                            hp, mc = h % 2, h // 2
                            lc = mc - 3 * g
                            for w in range(2):
                                nc.tensor.matmul(
                                    t_O[64 * w:64 * w + 64,
                                        64 * lh:64 * lh + 64],
                                    t_pt[64 * w:64 * w + 64,
                                         128 * lc + 64 * hp:128 * lc + 64 * hp + 64],
                                    t_v[64 * w:64 * w + 64, wp, 64 * h:64 * h + 64],
                                    start=True, stop=True,
                                    tile_position=(64 * w, 64 * w))
                        t_Osb = opool.tile([128, 384], DT)
                        nc.vector.tensor_copy(t_Osb, t_O)
                        # O^T: block b covers heads 6g+2b, 6g+2b+1 -> kc = 3g+b
                        if OT_DMA:
                            for b in range(3):
                                nc.sync.dma_start(
                                    out=t_ot[:, 3 * g + b, tb:tb + 128],
                                    in_=t_Osb[:, 128 * b:128 * b + 128],
                                    transpose=True)
                        else:
                            t_ot2 = psot.tile([128, 384], DT)
                            for b in range(3):
                                nc.tensor.transpose(
                                    t_ot2[:, 128 * b:128 * b + 128],
                                    t_Osb[:, 128 * b:128 * b + 128],
                                    t_id)
                            nc.vector.tensor_copy(
                                t_ot[:, 3 * g:3 * g + 3, tb:tb + 128],
                                t_ot2.rearrange("p (a b) -> p a b", a=3))

                # ---- output projection: out^T [pfeat, tok]
                t_out = outpool.tile([128, KC, CHTOK], DT)
                if skip_proj:
                    nc.vector.memset(t_out, 0.0)
                for mc in range(KC if not skip_proj else 0):
                    ps = psO.tile([128, CHTOK], F32, tag="opj")
                    for kc in range(KC):
                        nc.tensor.matmul(
                            ps, t_pw[:, kc, 128 * mc:128 * mc + 128],
                            t_ot[:, kc, :],
                            start=(kc == 0), stop=(kc == KC - 1))
                    nc.scalar.activation(
                        out=t_out[:, mc, :], in_=ps, func=AF.Identity,
                        bias=t_pb[:, mc:mc + 1], scale=1.0)
                nc.sync.dma_start(out=outr[:, :, c0:c0 + CHTOK], in_=t_out)

            loop_ctx = tc.For_i(0, rep, 1) if rep > 1 else contextlib.nullcontext()
            with loop_ctx:
                for ch in range(n_chunk):
                    chunk_body(ch)

    _split_multi_waits(nc)
    return nc


def _get_nc(safe_softmax=False):
    key = ("nc", safe_softmax, os.environ.get("KERNEL_DT", "fp16"),
           _qk_fp8(), os.environ.get("KERNEL_X8_RES", "1"))
    if key not in _CACHE:
        _CACHE[key] = _build(safe_softmax)
    return _CACHE[key]


def _prep_inputs(x, qkv_w, qkv_b, proj_w, proj_b, rpb_table, rel_pos_index):
    x = np.asarray(x, np.float32)
    qkv_w = np.asarray(qkv_w, np.float32)
    qkv_b = np.asarray(qkv_b, np.float32)
    proj_w = np.asarray(proj_w, np.float32)
    proj_b = np.asarray(proj_b, np.float32)
    rpb_table = np.asarray(rpb_table, np.float32)
    rel_pos_index = np.asarray(rel_pos_index)

    np_dt = _np_dt()
    QK8 = _qk_fp8()
    n8 = {"0": 0, "1": 12, "k": 6}[QK8]

    wqk = qkv_w[:, :2 * DIM].copy()
    # the attn scale is applied on the q activations device-side; the bias
    # is added after that scale, so fold it into the bias here
    bqk = qkv_b[:2 * DIM].copy()
    bqk[:DIM] *= SCALE
    wv = np.ascontiguousarray(qkv_w[:, 2 * DIM:])
    bv = qkv_b[2 * DIM:]
    pb_eff = proj_b + bv @ proj_w

    # rel-pos bias, gathered and laid out [row=(hp,n), col=(c,w,m)]
    bias_nmh = rpb_table[rel_pos_index]              # [n, m, h]
    bias_dup = np.empty((128, DIM), np.float32)
    for hp in range(2):
        for c in range(6):
            h = 2 * c + hp
            for w in range(2):
                bias_dup[64 * hp:64 * hp + 64,
                         128 * c + 64 * w:128 * c + 64 * w + 64] = bias_nmh[:, :, h]

    xT = np.ascontiguousarray(x.reshape(B * N, DIM).T)      # [768, 65536]
    wqk_blk = np.ascontiguousarray(
        wqk.reshape(KC, 128, 12, 128).transpose(2, 0, 1, 3))  # [mc, kc, p, m]
    common = {
        "wv": np.asarray(wv.astype(np_dt)),
        "pw": np.asarray(proj_w.astype(np_dt)),
        "bqk": np.ascontiguousarray(bqk.reshape(12, 128).T),
        "pb": np.ascontiguousarray(pb_eff.reshape(6, 128).T),
        "bias": np.asarray(bias_dup.astype(np_dt)),
        "ident": np.eye(128, dtype=np_dt),
        "identf": np.eye(128, dtype=np_dt),
    }
    X8RES = bool(int(os.environ.get("KERNEL_X8_RES", "1")))
    if n8:
        common["wqk8"] = np.asarray(
            (wqk_blk[12 - n8:] * W8SCALE).astype(ml_dtypes.float8_e4m3))
        x8T = np.asarray(xT.astype(ml_dtypes.float8_e4m3))
        if X8RES:
            x8bT = np.asarray(
                (xT - x8T.astype(np.float32)).astype(ml_dtypes.float8_e4m3))
    if n8 < 12:
        common["wqk"] = np.asarray(wqk_blk[:12 - n8].astype(np_dt))
    xTd = np.asarray(xT.astype(np_dt))
    in_maps = []
    for c in range(NCORES):
        m = dict(common)
        m["xT"] = np.ascontiguousarray(xTd[:, c * TOK:(c + 1) * TOK])
        if n8:
            m["x8T"] = np.ascontiguousarray(x8T[:, c * TOK:(c + 1) * TOK])
            if X8RES:
                m["x8bT"] = np.ascontiguousarray(
                    x8bT[:, c * TOK:(c + 1) * TOK])
        in_maps.append(m)
    return in_maps


def _run(nc, in_maps):
    res = run_bass_kernel_spmd(nc, in_maps, core_ids=list(range(NCORES)))
    out = np.empty((B * N, DIM), np.float32)
    for c in range(NCORES):
        out[c * TOK:(c + 1) * TOK] = res.results[c]["outT"].T.astype(np.float32)
    return out.reshape(B, N, DIM)


def kernel(x, qkv_w, qkv_b, proj_w, proj_b, rpb_table, rel_pos_index):
    in_maps = _prep_inputs(x, qkv_w, qkv_b, proj_w, proj_b,
                           rpb_table, rel_pos_index)
    out = _run(_get_nc(), in_maps)
    if not np.isfinite(out).all():
        # exp overflow/underflow (inputs far outside the reference scale):
        # rerun with the max-subtracted softmax variant
        out = _run(_get_nc(safe_softmax=True), in_maps)
    return out
